# revision 5
# baseline (speedup 1.0000x reference)
"""Block-diagonal 2x2 equalizer kernel for Trainium2 (8 NeuronCores), v2.2.

Per point (b, u, s, f) solves the 2x2 system M x = v by Cramer's rule:
    m_ij = h[b, pi[u], i, 0, 2u+j, s, f]   (only 1/4 of h is needed)
    det  = m00*m11 - m01*m10               (fp32: min |det| ~ 1.5e-4)
    x0   = (m11*v0 - m01*v1) / det         (numerators tolerate fp16)
    x1   = (m00*v1 - m10*v0) / det

Sharding: pure data parallel over batch, 2 batches per core on 8 cores.

Engine split per chunk (NCH=7, FC=256):
  DVE:  P = A*B wide fp32 {p0|p1}; det = P0-P1 narrow fp32;
        QA = Af*V wide fp16 (2x_1p), QB = Bf*Vsw wide fp16;
        R = {q0|q2}-{q1|q3} strided wide fp16; X = R*rdet_broadcast wide fp16
  ACT:  Af,Bf fp32->fp16 wide converts; Reciprocal spline fp32->fp16;
        also issues the chunk0/1 input DMAs (parallel with sync's 2..6)
  SYNC: input DMA issue + per-chunk output stores + final outS wait

Packing ([128, 1792] points/core chunked to [NCH, 128, FC]):
  A = {m11|m01}, B = {m00|m10}, V4 = {v0|v1|v1|v0}  (fp16, doubled so QB
  gets {v1|v0} as a contiguous slice)
    P  = A*B = {m11*m00 | m01*m10} = {p0|p1}
    QA = Af*V4[:, :WC]  = {q0|q1},  QB = Bf*V4[:, WC:] = {q2|q3}
    R  = {q0|q2} - {q1|q3} = {r0|r1}  (outer-strided APs on tQ)
    X  = R * rdet (stride-0 broadcast) = {x0|x1}
  One byte-packed input DMA per chunk (6144B descriptor rows); chunk 0 is
  split A+B / V4 so the fp32 chain starts ~0.6us earlier.
"""

from contextlib import ExitStack

import numpy as np

import concourse.bass as bass
import concourse.mybir as mybir
from concourse.bass_utils import run_bass_kernel_spmd

# Problem shapes (hardcoded per contract)
B, U, A, NTX, T, S, F = 16, 4, 2, 1, 8, 14, 2048
SF = S * F               # 28672
NCORES = 8
BPC = B // NCORES        # 2 batches per core
PTS = BPC * U * SF       # 229376 points per core
COLS = PTS // 128        # 1792
NCH = 7                  # pipeline chunks
FC = COLS // NCH         # 256
WC = 2 * FC              # 512
ABROW = 2 * WC * 4       # A+B fp32 bytes per partition row (4096)
VROW = 2 * WC * 2        # V4 fp16 bytes per partition row (2048)
ROW = ABROW + VROW       # 6144
D = 3                    # DVE fp16 lag (chunks)

TRACE = False
LAST_RESULTS = None

f32 = mybir.dt.float32
f16 = mybir.dt.float16
u8 = mybir.dt.uint8


def _build_nc():
    nc = bass.Bass("TRN2")
    dIn = nc.dram_tensor("dIn", [NCH, 128, ROW], u8, kind="ExternalInput")
    xO = nc.dram_tensor("xO", [NCH, 128, WC], f16, kind="ExternalOutput")

    with ExitStack() as ctx:
        sb = lambda name, w, dt: ctx.enter_context(nc.sbuf_tensor(name, [128, w], dt))
        tIn = [sb(f"tIn{k}", ROW, u8) for k in range(NCH)]
        tAf = [sb(f"tAf{k}", WC, f16) for k in range(NCH)]
        tBf = [sb(f"tBf{k}", WC, f16) for k in range(NCH)]
        tP = [sb(f"tP{k}", WC, f32) for k in range(NCH)]
        tDet = [sb(f"tDet{k}", FC, f32) for k in range(NCH)]
        tRd = [sb(f"tRd{k}", FC, f16) for k in range(NCH)]
        tQ = [sb(f"tQ{k}", 2 * WC, f16) for k in range(NCH)]
        tR = [sb(f"tR{k}", WC, f16) for k in range(NCH)]
        tX = sb("tX", NCH * WC, f16)

        vA = [tIn[k][:, 0:WC * 4].bitcast(f32) for k in range(NCH)]
        vB = [tIn[k][:, WC * 4:ABROW].bitcast(f32) for k in range(NCH)]
        vV = [tIn[k][:, ABROW:ROW].bitcast(f16) for k in range(NCH)]

        inS = [ctx.enter_context(nc.semaphore(f"inS{k}")) for k in range(NCH)]
        inV0 = ctx.enter_context(nc.semaphore("inV0"))
        dveS = ctx.enter_context(nc.semaphore("dveS"))
        actS = ctx.enter_context(nc.semaphore("actS"))
        outS = ctx.enter_context(nc.semaphore("outS"))

        det_idx = [0] * NCH
        x_idx = [0] * NCH
        recip_idx = [0] * NCH
        dc = 0
        for t in range(NCH + D):
            if t < NCH:
                dc += 2
                det_idx[t] = dc
            if t >= D:
                dc += 4
                x_idx[t - D] = dc
        ac = 0
        for t in range(NCH + 1):
            if 1 <= t <= NCH:
                ac += 1
                recip_idx[t - 1] = ac
            if t < NCH:
                ac += 2

        with nc.Block(no_gpsimd_drain=True) as block:

            @block.scalar
            def _(scalar):
                # chunk 0 split (A+B then V4) and chunk 1, on the qAct ring
                scalar.dma_start(out=tIn[0][:, :ABROW], in_=dIn[0, :, :ABROW]).then_inc(
                    inS[0], 16
                )
                scalar.dma_start(out=tIn[0][:, ABROW:], in_=dIn[0, :, ABROW:]).then_inc(
                    inV0, 16
                )
                scalar.dma_start(out=tIn[1][:], in_=dIn[1]).then_inc(inS[1], 16)
                for t in range(NCH + 1):
                    if 1 <= t <= NCH:
                        k = t - 1
                        scalar.wait_ge(dveS, det_idx[k])
                        scalar.add_instruction(
                            mybir.InstActivation(
                                name=nc.get_next_instruction_name(),
                                func=mybir.ActivationFunctionType.Reciprocal,
                                ins=[
                                    scalar.lower_ap(tDet[k][:]),
                                    mybir.ImmediateValue(dtype=f32, value=0.0),
                                    mybir.ImmediateValue(dtype=f32, value=1.0),
                                    mybir.ImmediateValue(dtype=f32, value=0.0),
                                ],
                                outs=[scalar.lower_ap(tRd[k][:])],
                            )
                        ).then_inc(actS, 1)
                    if t < NCH:
                        scalar.wait_ge(inS[t], 16)
                        scalar.copy(tAf[t][:], vA[t]).then_inc(actS, 1)
                        scalar.copy(tBf[t][:], vB[t]).then_inc(actS, 1)

            @block.sync
            def _(sync):
                for k in range(2, NCH):
                    sync.dma_start(out=tIn[k][:], in_=dIn[k]).then_inc(inS[k], 16)
                for k in range(NCH):
                    sync.wait_ge(dveS, x_idx[k])
                    sync.dma_start(
                        out=xO[k], in_=tX[:, k * WC:(k + 1) * WC]
                    ).then_inc(outS, 16)
                sync.wait_ge(outS, NCH * 16)

            @block.vector
            def _(vector):
                for t in range(NCH + D):
                    if t < NCH:
                        vector.wait_ge(inS[t], 16)
                        vector.tensor_mul(tP[t][:], vA[t], vB[t]).then_inc(dveS, 1)
                        vector.tensor_sub(
                            tDet[t][:], tP[t][:, :FC], tP[t][:, FC:]
                        ).then_inc(dveS, 1)
                    if t >= D:
                        k = t - D
                        q = tQ[k][:]
                        vector.wait_ge(actS, recip_idx[k])
                        if k == 0:
                            vector.wait_ge(inV0, 16)
                        vector.tensor_mul(q[:, :WC], tAf[k][:], vV[k][:, :WC]).then_inc(
                            dveS, 1
                        )
                        vector.tensor_mul(q[:, WC:], tBf[k][:], vV[k][:, WC:]).then_inc(
                            dveS, 1
                        )
                        q4 = q.rearrange("p (a c) -> p a c", a=4, c=FC)
                        rr = tR[k][:].rearrange("p (a c) -> p a c", a=2, c=FC)
                        vector.tensor_sub(rr, q4[:, 0::2], q4[:, 1::2]).then_inc(
                            dveS, 1
                        )
                        xx = tX[:, k * WC:(k + 1) * WC].rearrange(
                            "p (a c) -> p a c", a=2, c=FC
                        )
                        rdb = tRd[k][:].unsqueeze(1).broadcast_to((128, 2, FC))
                        vector.tensor_mul(xx, rr, rdb).then_inc(dveS, 1)

    return nc


def _chunk(plane):
    """[128*COLS] flat (C-order over [BPC,U,S,F]) -> [NCH, 128, FC]."""
    return plane.reshape(128, NCH, FC).transpose(1, 0, 2)


def make_in_maps(y, h, precoding_ind):
    """Host-side gather + byte-pack. Returns per-core input maps."""
    y = np.asarray(y)
    h = np.asarray(h)
    pi = np.asarray(precoding_ind).astype(np.int64)

    hg = h[:, pi[0]]                                     # [B, U, A, NTX, T, S, F]
    hsel = np.stack(
        [hg[:, u, :, 0, 2 * u:2 * u + 2] for u in range(U)], axis=1
    )                                                    # [B, U, A(i), 2(j), S, F]
    hsel = np.ascontiguousarray(hsel).astype(np.float32)
    yr = np.ascontiguousarray(y).astype(np.float32)      # [B, U, A, S, F]

    in_maps = []
    for c in range(NCORES):
        b0 = c * BPC
        hs = hsel[b0:b0 + BPC]
        ys = yr[b0:b0 + BPC]
        m00 = np.ascontiguousarray(hs[:, :, 0, 0]).reshape(-1)
        m01 = np.ascontiguousarray(hs[:, :, 0, 1]).reshape(-1)
        m10 = np.ascontiguousarray(hs[:, :, 1, 0]).reshape(-1)
        m11 = np.ascontiguousarray(hs[:, :, 1, 1]).reshape(-1)
        v0 = np.ascontiguousarray(ys[:, :, 0]).reshape(-1)
        v1 = np.ascontiguousarray(ys[:, :, 1]).reshape(-1)
        hA = np.concatenate([_chunk(m11), _chunk(m01)], axis=2)  # [NCH,128,WC] f32
        hB = np.concatenate([_chunk(m00), _chunk(m10)], axis=2)
        c0 = _chunk(v0).astype(np.float16)
        c1 = _chunk(v1).astype(np.float16)
        yV = np.concatenate([c0, c1, c1, c0], axis=2)            # [NCH,128,2*WC] f16
        dIn = np.concatenate(
            [
                hA.view(np.uint8).reshape(NCH, 128, WC * 4),
                hB.view(np.uint8).reshape(NCH, 128, WC * 4),
                yV.view(np.uint8).reshape(NCH, 128, VROW),
            ],
            axis=2,
        )
        in_maps.append({"dIn": np.ascontiguousarray(dIn)})
    return in_maps


def _unchunk(t):
    """[NCH, 128, FC] -> [128*COLS] flat."""
    return t.transpose(1, 0, 2).reshape(-1)


def assemble_output(results):
    """Per-core xO [NCH, 128, WC] f16 -> full [B, U, A, S, F] f32."""
    out = np.empty((B, U, A, S, F), np.float32)
    for c in range(NCORES):
        xo = np.asarray(results[c]["xO"]).astype(np.float32)
        x0 = _unchunk(xo[:, :, :FC]).reshape(BPC, U, S, F)
        x1 = _unchunk(xo[:, :, FC:]).reshape(BPC, U, S, F)
        out[c * BPC:(c + 1) * BPC, :, 0] = x0
        out[c * BPC:(c + 1) * BPC, :, 1] = x1
    return out


def kernel(y, h, precoding_ind):
    global LAST_RESULTS
    in_maps = make_in_maps(y, h, precoding_ind)
    nc = _build_nc()
    res = run_bass_kernel_spmd(nc, in_maps, list(range(NCORES)), trace=TRACE)
    LAST_RESULTS = res
    return assemble_output(res.results)


# revision 7
# speedup vs baseline: 1.1241x; 1.1241x over previous
"""Block-diagonal 2x2 equalizer kernel for Trainium2 (8 NeuronCores), v2.2.

Per point (b, u, s, f) solves the 2x2 system M x = v by Cramer's rule:
    m_ij = h[b, pi[u], i, 0, 2u+j, s, f]   (only 1/4 of h is needed)
    det  = m00*m11 - m01*m10               (fp32: min |det| ~ 1.5e-4)
    x0   = (m11*v0 - m01*v1) / det         (numerators tolerate fp16)
    x1   = (m00*v1 - m10*v0) / det

Sharding: pure data parallel over batch, 2 batches per core on 8 cores.

Engine split per chunk (NCH=7, FC=256):
  DVE:  P = A*B wide fp32 {p0|p1}; det = P0-P1 narrow fp32;
        QA = Af*V wide fp16 (2x_1p), QB = Bf*Vsw wide fp16;
        R = {q0|q2}-{q1|q3} strided wide fp16; X = R*rdet_broadcast wide fp16
  ACT:  Af,Bf fp32->fp16 wide converts; Reciprocal spline fp32->fp16;
        also issues the chunk0/1 input DMAs (parallel with sync's 2..6)
  SYNC: input DMA issue + per-chunk output stores + final outS wait

Packing ([128, 1792] points/core chunked to [NCH, 128, FC]):
  A = {m11|m01}, B = {m00|m10}, V4 = {v0|v1|v1|v0}  (fp16, doubled so QB
  gets {v1|v0} as a contiguous slice)
    P  = A*B = {m11*m00 | m01*m10} = {p0|p1}
    QA = Af*V4[:, :WC]  = {q0|q1},  QB = Bf*V4[:, WC:] = {q2|q3}
    R  = {q0|q2} - {q1|q3} = {r0|r1}  (outer-strided APs on tQ)
    X  = R * rdet (stride-0 broadcast) = {x0|x1}
  One byte-packed input DMA per chunk (6144B descriptor rows); chunk 0 is
  split A+B / V4 so the fp32 chain starts ~0.6us earlier.
"""

from contextlib import ExitStack

import numpy as np

import concourse.bass as bass
import concourse.mybir as mybir
from concourse.bass_utils import run_bass_kernel_spmd

# Problem shapes (hardcoded per contract)
B, U, A, NTX, T, S, F = 16, 4, 2, 1, 8, 14, 2048
SF = S * F               # 28672
NCORES = 8
BPC = B // NCORES        # 2 batches per core
PTS = BPC * U * SF       # 229376 points per core
COLS = PTS // 128        # 1792
NCH = 7                  # pipeline chunks
FC = COLS // NCH         # 256
WC = 2 * FC              # 512
ABROW = 2 * WC * 4       # A+B fp32 bytes per partition row (4096)
VROW = 2 * WC * 2        # V4 fp16 bytes per partition row (2048)
ROW = ABROW + VROW       # 6144
D = 3                    # DVE fp16 lag (chunks)

TRACE = False
LAST_RESULTS = None

f32 = mybir.dt.float32
f16 = mybir.dt.float16
u8 = mybir.dt.uint8


def _build_nc():
    nc = bass.Bass("TRN2")
    dIn = nc.dram_tensor("dIn", [NCH, 128, ROW], u8, kind="ExternalInput")
    xO = nc.dram_tensor("xO", [NCH, 128, WC], f16, kind="ExternalOutput")

    with ExitStack() as ctx:
        sb = lambda name, w, dt: ctx.enter_context(nc.sbuf_tensor(name, [128, w], dt))
        tIn = [sb(f"tIn{k}", ROW, u8) for k in range(NCH)]
        tAf = [sb(f"tAf{k}", WC, f16) for k in range(NCH)]
        tBf = [sb(f"tBf{k}", WC, f16) for k in range(NCH)]
        tP = [sb(f"tP{k}", WC, f32) for k in range(NCH)]
        tDet = [sb(f"tDet{k}", FC, f32) for k in range(NCH)]
        tRd = [sb(f"tRd{k}", FC, f16) for k in range(NCH)]
        tQ = [sb(f"tQ{k}", 2 * WC, f16) for k in range(NCH)]
        tR = [sb(f"tR{k}", WC, f16) for k in range(NCH)]
        tX = sb("tX", NCH * WC, f16)

        vA = [tIn[k][:, 0:WC * 4].bitcast(f32) for k in range(NCH)]
        vB = [tIn[k][:, WC * 4:ABROW].bitcast(f32) for k in range(NCH)]
        vV = [tIn[k][:, ABROW:ROW].bitcast(f16) for k in range(NCH)]

        inS = [ctx.enter_context(nc.semaphore(f"inS{k}")) for k in range(NCH)]
        inV0 = ctx.enter_context(nc.semaphore("inV0"))
        dveS = ctx.enter_context(nc.semaphore("dveS"))
        actS = ctx.enter_context(nc.semaphore("actS"))
        outS = ctx.enter_context(nc.semaphore("outS"))

        det_idx = [0] * NCH
        x_idx = [0] * NCH
        recip_idx = [0] * NCH
        dc = 0
        for t in range(NCH + D):
            if t < NCH:
                dc += 2
                det_idx[t] = dc
            if t >= D:
                dc += 4
                x_idx[t - D] = dc
        ac = 0
        for t in range(NCH + 1):
            if 1 <= t <= NCH:
                ac += 1
                recip_idx[t - 1] = ac
            if t < NCH:
                ac += 2

        with nc.Block(no_gpsimd_drain=True) as block:

            @block.scalar
            def _(scalar):
                for t in range(NCH + 1):
                    if 1 <= t <= NCH:
                        k = t - 1
                        scalar.wait_ge(dveS, det_idx[k])
                        scalar.add_instruction(
                            mybir.InstActivation(
                                name=nc.get_next_instruction_name(),
                                func=mybir.ActivationFunctionType.Reciprocal,
                                ins=[
                                    scalar.lower_ap(tDet[k][:]),
                                    mybir.ImmediateValue(dtype=f32, value=0.0),
                                    mybir.ImmediateValue(dtype=f32, value=1.0),
                                    mybir.ImmediateValue(dtype=f32, value=0.0),
                                ],
                                outs=[scalar.lower_ap(tRd[k][:])],
                            )
                        ).then_inc(actS, 1)
                    if t < NCH:
                        scalar.wait_ge(inS[t], 16)
                        scalar.copy(tAf[t][:], vA[t]).then_inc(actS, 1)
                        scalar.copy(tBf[t][:], vB[t]).then_inc(actS, 1)

            @block.sync
            def _(sync):
                # all inputs on one ring, in consumption order; chunk 0 split
                # so the fp32 chain starts as soon as A+B land
                sync.dma_start(out=tIn[0][:, :ABROW], in_=dIn[0, :, :ABROW]).then_inc(
                    inS[0], 16
                )
                sync.dma_start(out=tIn[0][:, ABROW:], in_=dIn[0, :, ABROW:]).then_inc(
                    inV0, 16
                )
                for k in range(1, NCH):
                    sync.dma_start(out=tIn[k][:], in_=dIn[k]).then_inc(inS[k], 16)
                for k in range(NCH):
                    sync.wait_ge(dveS, x_idx[k])
                    sync.dma_start(
                        out=xO[k], in_=tX[:, k * WC:(k + 1) * WC]
                    ).then_inc(outS, 16)
                sync.wait_ge(outS, NCH * 16)

            @block.vector
            def _(vector):
                for t in range(NCH + D):
                    if t < NCH:
                        vector.wait_ge(inS[t], 16)
                        vector.tensor_mul(tP[t][:], vA[t], vB[t]).then_inc(dveS, 1)
                        vector.tensor_sub(
                            tDet[t][:], tP[t][:, :FC], tP[t][:, FC:]
                        ).then_inc(dveS, 1)
                    if t >= D:
                        k = t - D
                        q = tQ[k][:]
                        vector.wait_ge(actS, recip_idx[k])
                        if k == 0:
                            vector.wait_ge(inV0, 16)
                        vector.tensor_mul(q[:, :WC], tAf[k][:], vV[k][:, :WC]).then_inc(
                            dveS, 1
                        )
                        vector.tensor_mul(q[:, WC:], tBf[k][:], vV[k][:, WC:]).then_inc(
                            dveS, 1
                        )
                        q4 = q.rearrange("p (a c) -> p a c", a=4, c=FC)
                        rr = tR[k][:].rearrange("p (a c) -> p a c", a=2, c=FC)
                        vector.tensor_sub(rr, q4[:, 0::2], q4[:, 1::2]).then_inc(
                            dveS, 1
                        )
                        xx = tX[:, k * WC:(k + 1) * WC].rearrange(
                            "p (a c) -> p a c", a=2, c=FC
                        )
                        rdb = tRd[k][:].unsqueeze(1).broadcast_to((128, 2, FC))
                        vector.tensor_mul(xx, rr, rdb).then_inc(dveS, 1)

    return nc


def _chunk(plane):
    """[128*COLS] flat (C-order over [BPC,U,S,F]) -> [NCH, 128, FC]."""
    return plane.reshape(128, NCH, FC).transpose(1, 0, 2)


def make_in_maps(y, h, precoding_ind):
    """Host-side gather + byte-pack. Returns per-core input maps."""
    y = np.asarray(y)
    h = np.asarray(h)
    pi = np.asarray(precoding_ind).astype(np.int64)

    hg = h[:, pi[0]]                                     # [B, U, A, NTX, T, S, F]
    hsel = np.stack(
        [hg[:, u, :, 0, 2 * u:2 * u + 2] for u in range(U)], axis=1
    )                                                    # [B, U, A(i), 2(j), S, F]
    hsel = np.ascontiguousarray(hsel).astype(np.float32)
    yr = np.ascontiguousarray(y).astype(np.float32)      # [B, U, A, S, F]

    in_maps = []
    for c in range(NCORES):
        b0 = c * BPC
        hs = hsel[b0:b0 + BPC]
        ys = yr[b0:b0 + BPC]
        m00 = np.ascontiguousarray(hs[:, :, 0, 0]).reshape(-1)
        m01 = np.ascontiguousarray(hs[:, :, 0, 1]).reshape(-1)
        m10 = np.ascontiguousarray(hs[:, :, 1, 0]).reshape(-1)
        m11 = np.ascontiguousarray(hs[:, :, 1, 1]).reshape(-1)
        v0 = np.ascontiguousarray(ys[:, :, 0]).reshape(-1)
        v1 = np.ascontiguousarray(ys[:, :, 1]).reshape(-1)
        hA = np.concatenate([_chunk(m11), _chunk(m01)], axis=2)  # [NCH,128,WC] f32
        hB = np.concatenate([_chunk(m00), _chunk(m10)], axis=2)
        c0 = _chunk(v0).astype(np.float16)
        c1 = _chunk(v1).astype(np.float16)
        yV = np.concatenate([c0, c1, c1, c0], axis=2)            # [NCH,128,2*WC] f16
        dIn = np.concatenate(
            [
                hA.view(np.uint8).reshape(NCH, 128, WC * 4),
                hB.view(np.uint8).reshape(NCH, 128, WC * 4),
                yV.view(np.uint8).reshape(NCH, 128, VROW),
            ],
            axis=2,
        )
        in_maps.append({"dIn": np.ascontiguousarray(dIn)})
    return in_maps


def _unchunk(t):
    """[NCH, 128, FC] -> [128*COLS] flat."""
    return t.transpose(1, 0, 2).reshape(-1)


def assemble_output(results):
    """Per-core xO [NCH, 128, WC] f16 -> full [B, U, A, S, F] f32."""
    out = np.empty((B, U, A, S, F), np.float32)
    for c in range(NCORES):
        xo = np.asarray(results[c]["xO"]).astype(np.float32)
        x0 = _unchunk(xo[:, :, :FC]).reshape(BPC, U, S, F)
        x1 = _unchunk(xo[:, :, FC:]).reshape(BPC, U, S, F)
        out[c * BPC:(c + 1) * BPC, :, 0] = x0
        out[c * BPC:(c + 1) * BPC, :, 1] = x1
    return out


def kernel(y, h, precoding_ind):
    global LAST_RESULTS
    in_maps = make_in_maps(y, h, precoding_ind)
    nc = _build_nc()
    res = run_bass_kernel_spmd(nc, in_maps, list(range(NCORES)), trace=TRACE)
    LAST_RESULTS = res
    return assemble_output(res.results)


# revision 8
# speedup vs baseline: 1.1905x; 1.0591x over previous
"""Block-diagonal 2x2 equalizer kernel for Trainium2 (8 NeuronCores), v2.5.

Per point (b, u, s, f) solves the 2x2 system M x = v by Cramer's rule:
    m_ij = h[b, pi[u], i, 0, 2u+j, s, f]   (only 1/4 of h is needed)
    det  = m00*m11 - m01*m10               (fp32: min |det| ~ 1.5e-4)
    x0   = (m11*v0 - m01*v1) / det         (numerators tolerate fp16)
    x1   = (m00*v1 - m10*v0) / det

Sharding: pure data parallel over batch, 2 batches per core on 8 cores.

Engine split per chunk (NCH=7, FC=256, WC=512):
  DVE (6 ops): P = A*B wide fp32 {p0|p1}; det = P0-P1;
       QQ = {Af|Bf} * Vbc  (one [128,1024] fp16 op at 2x_1p; Vbc is tV
       broadcast with a stride-0 outer dim) -> tQ = {q0|q1|q3|q2};
       R = {q0|q2}-{q1|q3} (strided); X = R*rdet_bc = {x0|x1}
  ACT: Af = cvt(A) wide; Bf = swapped cvt(B) 2 narrow ({m10f|m00f});
       Reciprocal spline fp32->fp16
  SYNC: 7 input DMAs in chunk order on one ring, then per-chunk stores,
       final outS wait.

Packing: A = {m11|m01}, B = {m00|m10}, V = {v0|v1}:
  P  = A*B = {m11*m00 | m01*m10} = {p0|p1}
  Af = {m11f|m01f},  Bf = {m10f|m00f}  (swap halves of B during convert)
  QQ = {Af|Bf}*{V|V} = {m11f*v0 | m01f*v1 | m10f*v0 | m00f*v1}
     = {q0|q1|q3|q2}
  R  = {q0|q2} - {q1|q3} = {r0|r1}   (outer strides 3FC / FC)
  X  = R * rdet = {x0|x1}
"""

from contextlib import ExitStack

import numpy as np

import concourse.bass as bass
import concourse.mybir as mybir
from concourse.bass_utils import run_bass_kernel_spmd

# Problem shapes (hardcoded per contract)
B, U, A, NTX, T, S, F = 16, 4, 2, 1, 8, 14, 2048
SF = S * F               # 28672
NCORES = 8
BPC = B // NCORES        # 2 batches per core
PTS = BPC * U * SF       # 229376 points per core
COLS = PTS // 128        # 1792
NCH = 7                  # pipeline chunks
FC = COLS // NCH         # 256
WC = 2 * FC              # 512
ABROW = 2 * WC * 4       # A+B fp32 bytes per partition row (4096)
VROW = WC * 2            # V fp16 bytes per partition row (1024)
ROW = ABROW + VROW       # 5120
D = 2                    # DVE fp16 lag (chunks)

TRACE = False
LAST_RESULTS = None

f32 = mybir.dt.float32
f16 = mybir.dt.float16
u8 = mybir.dt.uint8


def _build_nc():
    nc = bass.Bass("TRN2")
    dIn = nc.dram_tensor("dIn", [NCH, 128, ROW], u8, kind="ExternalInput")
    xO = nc.dram_tensor("xO", [NCH, 128, WC], f16, kind="ExternalOutput")

    with ExitStack() as ctx:
        sb = lambda name, w, dt: ctx.enter_context(nc.sbuf_tensor(name, [128, w], dt))
        tIn = [sb(f"tIn{k}", ROW, u8) for k in range(NCH)]
        tABf = [sb(f"tABf{k}", 2 * WC, f16) for k in range(NCH)]
        tP = [sb(f"tP{k}", WC, f32) for k in range(NCH)]
        tDet = [sb(f"tDet{k}", FC, f32) for k in range(NCH)]
        tRd = [sb(f"tRd{k}", FC, f16) for k in range(NCH)]
        tQ = [sb(f"tQ{k}", 2 * WC, f16) for k in range(NCH)]
        tR = [sb(f"tR{k}", WC, f16) for k in range(NCH)]
        tX = sb("tX", NCH * WC, f16)

        vA = [tIn[k][:, 0:WC * 4].bitcast(f32) for k in range(NCH)]
        vB = [tIn[k][:, WC * 4:ABROW].bitcast(f32) for k in range(NCH)]
        vV = [tIn[k][:, ABROW:ROW].bitcast(f16) for k in range(NCH)]

        inS = [ctx.enter_context(nc.semaphore(f"inS{k}")) for k in range(NCH)]
        dveS = ctx.enter_context(nc.semaphore("dveS"))
        actS = ctx.enter_context(nc.semaphore("actS"))
        outS = ctx.enter_context(nc.semaphore("outS"))

        det_idx = [0] * NCH
        x_idx = [0] * NCH
        recip_idx = [0] * NCH
        dc = 0
        for t in range(NCH + D):
            if t < NCH:
                dc += 2
                det_idx[t] = dc
            if t >= D:
                dc += 3
                x_idx[t - D] = dc
        ac = 0
        for t in range(NCH + 1):
            if 1 <= t <= NCH:
                ac += 1
                recip_idx[t - 1] = ac
            if t < NCH:
                ac += 3

        with nc.Block(no_gpsimd_drain=True) as block:

            @block.scalar
            def _(scalar):
                for t in range(NCH + 1):
                    if 1 <= t <= NCH:
                        k = t - 1
                        scalar.wait_ge(dveS, det_idx[k])
                        scalar.add_instruction(
                            mybir.InstActivation(
                                name=nc.get_next_instruction_name(),
                                func=mybir.ActivationFunctionType.Reciprocal,
                                ins=[
                                    scalar.lower_ap(tDet[k][:]),
                                    mybir.ImmediateValue(dtype=f32, value=0.0),
                                    mybir.ImmediateValue(dtype=f32, value=1.0),
                                    mybir.ImmediateValue(dtype=f32, value=0.0),
                                ],
                                outs=[scalar.lower_ap(tRd[k][:])],
                            )
                        ).then_inc(actS, 1)
                    if t < NCH:
                        scalar.wait_ge(inS[t], 16)
                        scalar.copy(tABf[t][:, :WC], vA[t]).then_inc(actS, 1)
                        scalar.copy(tABf[t][:, WC:WC + FC], vB[t][:, FC:]).then_inc(
                            actS, 1
                        )
                        scalar.copy(tABf[t][:, WC + FC:], vB[t][:, :FC]).then_inc(
                            actS, 1
                        )

            @block.sync
            def _(sync):
                for k in range(NCH):
                    sync.dma_start(out=tIn[k][:], in_=dIn[k]).then_inc(inS[k], 16)
                for k in range(NCH):
                    sync.wait_ge(dveS, x_idx[k])
                    sync.dma_start(
                        out=xO[k], in_=tX[:, k * WC:(k + 1) * WC]
                    ).then_inc(outS, 16)
                sync.wait_ge(outS, NCH * 16)

            @block.vector
            def _(vector):
                for t in range(NCH + D):
                    if t < NCH:
                        vector.wait_ge(inS[t], 16)
                        vector.tensor_mul(tP[t][:], vA[t], vB[t]).then_inc(dveS, 1)
                        vector.tensor_sub(
                            tDet[t][:], tP[t][:, :FC], tP[t][:, FC:]
                        ).then_inc(dveS, 1)
                    if t >= D:
                        k = t - D
                        vector.wait_ge(actS, recip_idx[k])
                        qq = tQ[k][:].rearrange("p (a c) -> p a c", a=2, c=WC)
                        vbc = vV[k].unsqueeze(1).broadcast_to((128, 2, WC))
                        abf = tABf[k][:].rearrange("p (a c) -> p a c", a=2, c=WC)
                        vector.tensor_mul(qq, abf, vbc).then_inc(dveS, 1)
                        q4 = tQ[k][:].rearrange("p (a c) -> p a c", a=4, c=FC)
                        rr = tR[k][:].rearrange("p (a c) -> p a c", a=2, c=FC)
                        vector.tensor_sub(rr, q4[:, 0::3], q4[:, 1:3]).then_inc(
                            dveS, 1
                        )
                        xx = tX[:, k * WC:(k + 1) * WC].rearrange(
                            "p (a c) -> p a c", a=2, c=FC
                        )
                        rdb = tRd[k][:].unsqueeze(1).broadcast_to((128, 2, FC))
                        vector.tensor_mul(xx, rr, rdb).then_inc(dveS, 1)

    return nc


def _chunk(plane):
    """[128*COLS] flat (C-order over [BPC,U,S,F]) -> [NCH, 128, FC]."""
    return plane.reshape(128, NCH, FC).transpose(1, 0, 2)


def make_in_maps(y, h, precoding_ind):
    """Host-side gather + byte-pack. Returns per-core input maps."""
    y = np.asarray(y)
    h = np.asarray(h)
    pi = np.asarray(precoding_ind).astype(np.int64)

    hg = h[:, pi[0]]                                     # [B, U, A, NTX, T, S, F]
    hsel = np.stack(
        [hg[:, u, :, 0, 2 * u:2 * u + 2] for u in range(U)], axis=1
    )                                                    # [B, U, A(i), 2(j), S, F]
    hsel = np.ascontiguousarray(hsel).astype(np.float32)
    yr = np.ascontiguousarray(y).astype(np.float32)      # [B, U, A, S, F]

    in_maps = []
    for c in range(NCORES):
        b0 = c * BPC
        hs = hsel[b0:b0 + BPC]
        ys = yr[b0:b0 + BPC]
        m00 = np.ascontiguousarray(hs[:, :, 0, 0]).reshape(-1)
        m01 = np.ascontiguousarray(hs[:, :, 0, 1]).reshape(-1)
        m10 = np.ascontiguousarray(hs[:, :, 1, 0]).reshape(-1)
        m11 = np.ascontiguousarray(hs[:, :, 1, 1]).reshape(-1)
        v0 = np.ascontiguousarray(ys[:, :, 0]).reshape(-1)
        v1 = np.ascontiguousarray(ys[:, :, 1]).reshape(-1)
        hA = np.concatenate([_chunk(m11), _chunk(m01)], axis=2)  # [NCH,128,WC] f32
        hB = np.concatenate([_chunk(m00), _chunk(m10)], axis=2)
        yV = np.concatenate(
            [_chunk(v0).astype(np.float16), _chunk(v1).astype(np.float16)], axis=2
        )
        dIn = np.concatenate(
            [
                hA.view(np.uint8).reshape(NCH, 128, WC * 4),
                hB.view(np.uint8).reshape(NCH, 128, WC * 4),
                yV.view(np.uint8).reshape(NCH, 128, VROW),
            ],
            axis=2,
        )
        in_maps.append({"dIn": np.ascontiguousarray(dIn)})
    return in_maps


def _unchunk(t):
    """[NCH, 128, FC] -> [128*COLS] flat."""
    return t.transpose(1, 0, 2).reshape(-1)


def assemble_output(results):
    """Per-core xO [NCH, 128, WC] f16 -> full [B, U, A, S, F] f32."""
    out = np.empty((B, U, A, S, F), np.float32)
    for c in range(NCORES):
        xo = np.asarray(results[c]["xO"]).astype(np.float32)
        x0 = _unchunk(xo[:, :, :FC]).reshape(BPC, U, S, F)
        x1 = _unchunk(xo[:, :, FC:]).reshape(BPC, U, S, F)
        out[c * BPC:(c + 1) * BPC, :, 0] = x0
        out[c * BPC:(c + 1) * BPC, :, 1] = x1
    return out


def kernel(y, h, precoding_ind):
    global LAST_RESULTS
    in_maps = make_in_maps(y, h, precoding_ind)
    nc = _build_nc()
    res = run_bass_kernel_spmd(nc, in_maps, list(range(NCORES)), trace=TRACE)
    LAST_RESULTS = res
    return assemble_output(res.results)


# revision 9
# speedup vs baseline: 1.2536x; 1.0530x over previous
"""Block-diagonal 2x2 equalizer kernel for Trainium2 (8 NeuronCores), v2.5.

Per point (b, u, s, f) solves the 2x2 system M x = v by Cramer's rule:
    m_ij = h[b, pi[u], i, 0, 2u+j, s, f]   (only 1/4 of h is needed)
    det  = m00*m11 - m01*m10               (fp32: min |det| ~ 1.5e-4)
    x0   = (m11*v0 - m01*v1) / det         (numerators tolerate fp16)
    x1   = (m00*v1 - m10*v0) / det

Sharding: pure data parallel over batch, 2 batches per core on 8 cores.

Engine split per chunk (NCH=7, FC=256, WC=512):
  DVE (6 ops): P = A*B wide fp32 {p0|p1}; det = P0-P1;
       QQ = {Af|Bf} * Vbc  (one [128,1024] fp16 op at 2x_1p; Vbc is tV
       broadcast with a stride-0 outer dim) -> tQ = {q0|q1|q3|q2};
       R = {q0|q2}-{q1|q3} (strided); X = R*rdet_bc = {x0|x1}
  ACT: Af = cvt(A) wide; Bf = swapped cvt(B) 2 narrow ({m10f|m00f});
       Reciprocal spline fp32->fp16
  SYNC: 7 input DMAs in chunk order on one ring, then per-chunk stores,
       final outS wait.

Packing: A = {m11|m01}, B = {m00|m10}, V = {v0|v1}:
  P  = A*B = {m11*m00 | m01*m10} = {p0|p1}
  Af = {m11f|m01f},  Bf = {m10f|m00f}  (swap halves of B during convert)
  QQ = {Af|Bf}*{V|V} = {m11f*v0 | m01f*v1 | m10f*v0 | m00f*v1}
     = {q0|q1|q3|q2}
  R  = {q0|q2} - {q1|q3} = {r0|r1}   (outer strides 3FC / FC)
  X  = R * rdet = {x0|x1}
"""

from contextlib import ExitStack

import numpy as np

import concourse.bass as bass
import concourse.mybir as mybir
from concourse.bass_utils import run_bass_kernel_spmd

# Problem shapes (hardcoded per contract)
B, U, A, NTX, T, S, F = 16, 4, 2, 1, 8, 14, 2048
SF = S * F               # 28672
NCORES = 8
BPC = B // NCORES        # 2 batches per core
PTS = BPC * U * SF       # 229376 points per core
COLS = PTS // 128        # 1792
NCH = 8                  # pipeline chunks
FC = COLS // NCH         # 256
WC = 2 * FC              # 512
ABROW = 2 * WC * 4       # A+B fp32 bytes per partition row (4096)
VROW = WC * 2            # V fp16 bytes per partition row (1024)
ROW = ABROW + VROW       # 5120
D = 1                    # DVE fp16 lag (chunks)

TRACE = False
LAST_RESULTS = None

f32 = mybir.dt.float32
f16 = mybir.dt.float16
u8 = mybir.dt.uint8


def _build_nc():
    nc = bass.Bass("TRN2")
    dIn = nc.dram_tensor("dIn", [NCH, 128, ROW], u8, kind="ExternalInput")
    xO = nc.dram_tensor("xO", [NCH, 128, WC], f16, kind="ExternalOutput")

    with ExitStack() as ctx:
        sb = lambda name, w, dt: ctx.enter_context(nc.sbuf_tensor(name, [128, w], dt))
        tIn = [sb(f"tIn{k}", ROW, u8) for k in range(NCH)]
        tABf = [sb(f"tABf{k}", 2 * WC, f16) for k in range(NCH)]
        tP = [sb(f"tP{k}", WC, f32) for k in range(NCH)]
        tDet = [sb(f"tDet{k}", FC, f32) for k in range(NCH)]
        tRd = [sb(f"tRd{k}", FC, f16) for k in range(NCH)]
        tQ = [sb(f"tQ{k}", 2 * WC, f16) for k in range(NCH)]
        tR = [sb(f"tR{k}", WC, f16) for k in range(NCH)]
        tX = sb("tX", NCH * WC, f16)

        vA = [tIn[k][:, 0:WC * 4].bitcast(f32) for k in range(NCH)]
        vB = [tIn[k][:, WC * 4:ABROW].bitcast(f32) for k in range(NCH)]
        vV = [tIn[k][:, ABROW:ROW].bitcast(f16) for k in range(NCH)]

        inS = [ctx.enter_context(nc.semaphore(f"inS{k}")) for k in range(NCH)]
        dveS = ctx.enter_context(nc.semaphore("dveS"))
        actS = ctx.enter_context(nc.semaphore("actS"))
        outS = ctx.enter_context(nc.semaphore("outS"))

        det_idx = [0] * NCH
        x_idx = [0] * NCH
        recip_idx = [0] * NCH
        dc = 0
        for t in range(NCH + D):
            if t < NCH:
                dc += 2
                det_idx[t] = dc
            if t >= D:
                dc += 3
                x_idx[t - D] = dc
        ac = 0
        for t in range(NCH + 1):
            if 1 <= t <= NCH:
                ac += 1
                recip_idx[t - 1] = ac
            if t < NCH:
                ac += 3

        with nc.Block(no_gpsimd_drain=True) as block:

            @block.scalar
            def _(scalar):
                for t in range(NCH + 1):
                    if 1 <= t <= NCH:
                        k = t - 1
                        scalar.wait_ge(dveS, det_idx[k])
                        scalar.add_instruction(
                            mybir.InstActivation(
                                name=nc.get_next_instruction_name(),
                                func=mybir.ActivationFunctionType.Reciprocal,
                                ins=[
                                    scalar.lower_ap(tDet[k][:]),
                                    mybir.ImmediateValue(dtype=f32, value=0.0),
                                    mybir.ImmediateValue(dtype=f32, value=1.0),
                                    mybir.ImmediateValue(dtype=f32, value=0.0),
                                ],
                                outs=[scalar.lower_ap(tRd[k][:])],
                            )
                        ).then_inc(actS, 1)
                    if t < NCH:
                        scalar.wait_ge(inS[t], 16)
                        scalar.copy(tABf[t][:, :WC], vA[t]).then_inc(actS, 1)
                        scalar.copy(tABf[t][:, WC:WC + FC], vB[t][:, FC:]).then_inc(
                            actS, 1
                        )
                        scalar.copy(tABf[t][:, WC + FC:], vB[t][:, :FC]).then_inc(
                            actS, 1
                        )

            @block.sync
            def _(sync):
                for k in range(NCH):
                    sync.dma_start(out=tIn[k][:], in_=dIn[k]).then_inc(inS[k], 16)
                for k in range(NCH):
                    sync.wait_ge(dveS, x_idx[k])
                    sync.dma_start(
                        out=xO[k], in_=tX[:, k * WC:(k + 1) * WC]
                    ).then_inc(outS, 16)
                sync.wait_ge(outS, NCH * 16)

            @block.vector
            def _(vector):
                for t in range(NCH + D):
                    if t < NCH:
                        vector.wait_ge(inS[t], 16)
                        vector.tensor_mul(tP[t][:], vA[t], vB[t]).then_inc(dveS, 1)
                        vector.tensor_sub(
                            tDet[t][:], tP[t][:, :FC], tP[t][:, FC:]
                        ).then_inc(dveS, 1)
                    if t >= D:
                        k = t - D
                        vector.wait_ge(actS, recip_idx[k])
                        qq = tQ[k][:].rearrange("p (a c) -> p a c", a=2, c=WC)
                        vbc = vV[k].unsqueeze(1).broadcast_to((128, 2, WC))
                        abf = tABf[k][:].rearrange("p (a c) -> p a c", a=2, c=WC)
                        vector.tensor_mul(qq, abf, vbc).then_inc(dveS, 1)
                        q4 = tQ[k][:].rearrange("p (a c) -> p a c", a=4, c=FC)
                        rr = tR[k][:].rearrange("p (a c) -> p a c", a=2, c=FC)
                        vector.tensor_sub(rr, q4[:, 0::3], q4[:, 1:3]).then_inc(
                            dveS, 1
                        )
                        xx = tX[:, k * WC:(k + 1) * WC].rearrange(
                            "p (a c) -> p a c", a=2, c=FC
                        )
                        rdb = tRd[k][:].unsqueeze(1).broadcast_to((128, 2, FC))
                        vector.tensor_mul(xx, rr, rdb).then_inc(dveS, 1)

    return nc


def _chunk(plane):
    """[128*COLS] flat (C-order over [BPC,U,S,F]) -> [NCH, 128, FC]."""
    return plane.reshape(128, NCH, FC).transpose(1, 0, 2)


def make_in_maps(y, h, precoding_ind):
    """Host-side gather + byte-pack. Returns per-core input maps."""
    y = np.asarray(y)
    h = np.asarray(h)
    pi = np.asarray(precoding_ind).astype(np.int64)

    hg = h[:, pi[0]]                                     # [B, U, A, NTX, T, S, F]
    hsel = np.stack(
        [hg[:, u, :, 0, 2 * u:2 * u + 2] for u in range(U)], axis=1
    )                                                    # [B, U, A(i), 2(j), S, F]
    hsel = np.ascontiguousarray(hsel).astype(np.float32)
    yr = np.ascontiguousarray(y).astype(np.float32)      # [B, U, A, S, F]

    in_maps = []
    for c in range(NCORES):
        b0 = c * BPC
        hs = hsel[b0:b0 + BPC]
        ys = yr[b0:b0 + BPC]
        m00 = np.ascontiguousarray(hs[:, :, 0, 0]).reshape(-1)
        m01 = np.ascontiguousarray(hs[:, :, 0, 1]).reshape(-1)
        m10 = np.ascontiguousarray(hs[:, :, 1, 0]).reshape(-1)
        m11 = np.ascontiguousarray(hs[:, :, 1, 1]).reshape(-1)
        v0 = np.ascontiguousarray(ys[:, :, 0]).reshape(-1)
        v1 = np.ascontiguousarray(ys[:, :, 1]).reshape(-1)
        hA = np.concatenate([_chunk(m11), _chunk(m01)], axis=2)  # [NCH,128,WC] f32
        hB = np.concatenate([_chunk(m00), _chunk(m10)], axis=2)
        yV = np.concatenate(
            [_chunk(v0).astype(np.float16), _chunk(v1).astype(np.float16)], axis=2
        )
        dIn = np.concatenate(
            [
                hA.view(np.uint8).reshape(NCH, 128, WC * 4),
                hB.view(np.uint8).reshape(NCH, 128, WC * 4),
                yV.view(np.uint8).reshape(NCH, 128, VROW),
            ],
            axis=2,
        )
        in_maps.append({"dIn": np.ascontiguousarray(dIn)})
    return in_maps


def _unchunk(t):
    """[NCH, 128, FC] -> [128*COLS] flat."""
    return t.transpose(1, 0, 2).reshape(-1)


def assemble_output(results):
    """Per-core xO [NCH, 128, WC] f16 -> full [B, U, A, S, F] f32."""
    out = np.empty((B, U, A, S, F), np.float32)
    for c in range(NCORES):
        xo = np.asarray(results[c]["xO"]).astype(np.float32)
        x0 = _unchunk(xo[:, :, :FC]).reshape(BPC, U, S, F)
        x1 = _unchunk(xo[:, :, FC:]).reshape(BPC, U, S, F)
        out[c * BPC:(c + 1) * BPC, :, 0] = x0
        out[c * BPC:(c + 1) * BPC, :, 1] = x1
    return out


def kernel(y, h, precoding_ind):
    global LAST_RESULTS
    in_maps = make_in_maps(y, h, precoding_ind)
    nc = _build_nc()
    res = run_bass_kernel_spmd(nc, in_maps, list(range(NCORES)), trace=TRACE)
    LAST_RESULTS = res
    return assemble_output(res.results)
